# revision 8
# baseline (speedup 1.0000x reference)
"""AutoEncoderTopK (B=4096, D=2048, F=32768, K=64) on 8 trn2 NeuronCores.

Data-parallel over batch (512 rows/core). Per core:
  P1  encode pre = (x - b_dec) @ W_enc.T + b_enc with the bf16x3 scheme
      (x_hi*w_hi + x_hi*w_lo + x_lo*w_hi, fp32 PSUM accumulate; ~1e-5 abs
      accurate -- enough for exact top-k selection at this distribution);
      relu fused into PSUM eviction; fp32 pre spilled to DRAM scratch;
      per-256-feature-chunk top-8 screen (DVE max8) -> 1024 candidates/row.
  P2  8 rounds of max8 + match_replace over candidates -> exact 64th largest
      value t* per row (values-only top-k; no indices/scatter needed).
  P3  per block-pair: re-stream pre, masked = (pre >= t*) * pre as bf16
      (split across DVE and GpSimd), PE-transpose into SBUF-resident
      maskedT half-buffers per block.
  P4  per block-pair: decode x_hat = masked @ W_dec.T + b_dec, one pass over
      W_dec.T per pair (2 blocks x 4 d-tiles = 8 PSUM accumulators).
"""

import sys

sys.path.insert(0, "/opt/trn_rl_repo")

import numpy as np
import ml_dtypes

import concourse.bacc as bacc
import concourse.mybir as mybir
import concourse.tile as tile
from concourse.bass_utils import run_bass_kernel_spmd
from concourse.masks import make_identity

F32 = mybir.dt.float32
BF16 = mybir.dt.bfloat16

B, D, F, K = 4096, 2048, 32768, 64
NCORES = 8
BC = B // NCORES            # 512 rows per core
NB = BC // 128              # 4 blocks of 128 rows
FT = F // 512               # 64 f-tiles of 512 features
NFC = F // 128              # 256 f-chunks of 128 features (decode)
CHUNK = 256                 # screening chunk
NCH = 512 // CHUNK          # max8 calls per f-tile
CAND = FT * NCH * 8         # 1024 candidates per row
DT = D // 512               # 4 d-tiles of 512 (decode output)
HFC = NFC // 2              # f-chunks per maskedT half

_cache = {}


def _build(dc):
    """dc = number of 128-row contraction chunks in encode (16, or 17 when
    b_enc is folded in via an augmented ones-row chunk)."""
    nc = bacc.Bacc()
    xh = nc.dram_tensor("xh", [dc * 128, BC], BF16, kind="ExternalInput")
    xl = nc.dram_tensor("xl", [dc * 128, BC], BF16, kind="ExternalInput")
    weh = nc.dram_tensor("weh", [dc * 128, F], BF16, kind="ExternalInput")
    wel = nc.dram_tensor("wel", [dc * 128, F], BF16, kind="ExternalInput")
    wdt = nc.dram_tensor("wdt", [F + 128, D], BF16, kind="ExternalInput")
    xhat = nc.dram_tensor("xhat", [BC, D], F32, kind="ExternalOutput")
    pre_dram = nc.dram_tensor("pre_scratch", [BC, F], F32, kind="Internal")

    with tile.TileContext(nc) as tc:
        with (
            tc.tile_pool(name="const", bufs=1) as cpool,
            tc.tile_pool(name="w", bufs=8) as wpool,
            tc.tile_pool(name="pre", bufs=4) as prepool,
            tc.tile_pool(name="small", bufs=4) as spool,
            tc.tile_pool(name="ps", bufs=8, space="PSUM") as ps,
        ):
            ident = cpool.tile([128, 128], BF16, tag="ident")
            make_identity(nc, ident)
            ones_row = cpool.tile([128, 128], BF16, tag="ones_row")
            nc.vector.memset(ones_row, 0.0)
            nc.vector.memset(ones_row[0:1, :], 1.0)
            cands = [
                cpool.tile([128, CAND], F32, tag=f"cand{b}", name=f"cand{b}")
                for b in range(NB)
            ]
            tstars = [
                cpool.tile([128, 1], F32, tag=f"tstar{b}", name=f"tstar{b}")
                for b in range(NB)
            ]

            # ---- P1: encode (bf16x3) + screen ----
            with tc.tile_pool(name="xtp", bufs=1) as xtpool:
                xh_c = []
                xl_c = []
                for c in range(dc):
                    th = xtpool.tile([128, BC], BF16, tag=f"xh{c}", name=f"xh{c}")
                    nc.sync.dma_start(th, xh[c * 128 : (c + 1) * 128, :])
                    xh_c.append(th)
                    tl = xtpool.tile([128, BC], BF16, tag=f"xl{c}", name=f"xl{c}")
                    nc.sync.dma_start(tl, xl[c * 128 : (c + 1) * 128, :])
                    xl_c.append(tl)
                for ft in range(FT):
                    psum_e = [
                        ps.tile([128, 512], F32, tag="acc", name=f"pse{ft}_{i}")
                        for i in range(NB)
                    ]
                    for c in range(dc):
                        wht = wpool.tile([128, 512], BF16, tag="w", name="wht")
                        nc.sync.dma_start(
                            wht,
                            weh[c * 128 : (c + 1) * 128, ft * 512 : (ft + 1) * 512],
                        )
                        wlt = wpool.tile([128, 512], BF16, tag="w", name="wlt")
                        nc.sync.dma_start(
                            wlt,
                            wel[c * 128 : (c + 1) * 128, ft * 512 : (ft + 1) * 512],
                        )
                        for blk in range(NB):
                            lh = xh_c[c][:, blk * 128 : (blk + 1) * 128]
                            ll = xl_c[c][:, blk * 128 : (blk + 1) * 128]
                            nc.tensor.matmul(
                                psum_e[blk], lh, wht, start=(c == 0), stop=False
                            )
                            nc.tensor.matmul(
                                psum_e[blk], lh, wlt, start=False, stop=False
                            )
                            nc.tensor.matmul(
                                psum_e[blk], ll, wht, start=False,
                                stop=(c == dc - 1),
                            )
                    for blk in range(NB):
                        pre_t = prepool.tile([128, 512], F32, tag="pre")
                        nc.scalar.activation(
                            pre_t, psum_e[blk], mybir.ActivationFunctionType.Relu
                        )
                        nc.sync.dma_start(
                            pre_dram[
                                blk * 128 : (blk + 1) * 128,
                                ft * 512 : (ft + 1) * 512,
                            ],
                            pre_t,
                        )
                        for j in range(NCH):
                            nc.vector.max(
                                out=cands[blk][
                                    :, (ft * NCH + j) * 8 : (ft * NCH + j) * 8 + 8
                                ],
                                in_=pre_t[:, j * CHUNK : (j + 1) * CHUNK],
                            )

            def emit_rounds(blk):
                # t* = 64th largest via 8 rounds of max8 + match_replace
                for r in range(K // 8):
                    m8 = spool.tile([128, 8], F32, tag="m8", name=f"m8_{blk}_{r}")
                    nc.vector.max(out=m8, in_=cands[blk])
                    if r < K // 8 - 1:
                        nc.vector.match_replace(
                            out=cands[blk],
                            in_to_replace=m8,
                            in_values=cands[blk],
                            imm_value=-1e30,
                        )
                    else:
                        nc.vector.tensor_copy(tstars[blk], m8[:, 7:8])

            # ---- P3 + P4 per block pair ----
            with (
                tc.tile_pool(name="mtp", bufs=1) as mtpool,
                tc.tile_pool(name="dec", bufs=6) as decpool,
                tc.tile_pool(name="pref", bufs=5) as prefpool,
            ):
                # maskedT half-buffers: [bi][half] so pair N+1's transposes
                # into one half overlap pair N's decode reads of the other
                mts = [
                    [
                        mtpool.tile(
                            [128, HFC * 128], BF16,
                            tag=f"mt{i}_{h}", name=f"mt{i}_{h}",
                        )
                        for h in range(2)
                    ]
                    for i in range(2)
                ]
                for pg in range(NB // 2):
                    pair = (2 * pg, 2 * pg + 1)
                    for blk in pair:
                        emit_rounds(blk)
                    psd = [
                        [
                            ps.tile(
                                [128, 512], F32, tag="acc", name=f"psd{pg}_{i}_{d}"
                            )
                            for d in range(2)
                        ]
                        for i in range(2)
                    ]
                    wa = decpool.tile([128, D], BF16, tag="wa", name="wa", bufs=1)
                    nc.sync.dma_start(wa, wdt[F : F + 128, :])
                    for dh in range(2):
                        # bias chunk opens each accumulation group
                        for bi in range(2):
                            for dt_i in range(2):
                                nc.tensor.matmul(
                                    psd[bi][dt_i],
                                    ones_row,
                                    wa[
                                        :,
                                        dh * 1024 + dt_i * 512 :
                                        dh * 1024 + (dt_i + 1) * 512,
                                    ],
                                    start=True,
                                    stop=False,
                                )
                        for fc in range(NFC):
                            h, fcl = fc // HFC, fc % HFC
                            if dh == 0 and fc % 4 == 0:
                                # produce masked bf16 + transposes for this f-tile
                                ft = fc // 4
                                for bi, blk in enumerate(pair):
                                    pre_t = prefpool.tile(
                                        [128, 512], F32, tag="pref",
                                        name=f"pref{bi}",
                                    )
                                    nc.sync.dma_start(
                                        pre_t,
                                        pre_dram[
                                            blk * 128 : (blk + 1) * 128,
                                            ft * 512 : (ft + 1) * 512,
                                        ],
                                    )
                                    msk = prefpool.tile(
                                        [128, 512], BF16, tag="mskf",
                                        name=f"mskf{bi}",
                                    )
                                    nc.vector.scalar_tensor_tensor(
                                        out=msk,
                                        in0=pre_t,
                                        scalar=tstars[blk][:, 0:1],
                                        in1=pre_t,
                                        op0=mybir.AluOpType.is_ge,
                                        op1=mybir.AluOpType.mult,
                                    )
                                    for j in range(4):
                                        fcj = ft * 4 + j
                                        hj, fclj = fcj // HFC, fcj % HFC
                                        pt = ps.tile(
                                            [128, 128], BF16, tag="acc", name="pt"
                                        )
                                        nc.tensor.transpose(
                                            pt,
                                            msk[:, j * 128 : (j + 1) * 128],
                                            ident,
                                        )
                                        nc.any.tensor_copy(
                                            mts[bi][hj][
                                                :, fclj * 128 : (fclj + 1) * 128
                                            ],
                                            pt,
                                        )
                            wd = decpool.tile(
                                [128, 1024], BF16, tag="wd", name="wd"
                            )
                            nc.sync.dma_start(
                                wd,
                                wdt[
                                    fc * 128 : (fc + 1) * 128,
                                    dh * 1024 : (dh + 1) * 1024,
                                ],
                            )
                            last = fc == NFC - 1
                            for bi in range(2):
                                lhsT = mts[bi][h][:, fcl * 128 : (fcl + 1) * 128]
                                for dt_i in range(2):
                                    nc.tensor.matmul(
                                        psd[bi][dt_i],
                                        lhsT,
                                        wd[:, dt_i * 512 : (dt_i + 1) * 512],
                                        start=False,
                                        stop=last,
                                    )
                        for bi, blk in enumerate(pair):
                            for dt_i in range(2):
                                o = prepool.tile([128, 512], F32, tag="o")
                                nc.scalar.activation(
                                    o,
                                    psd[bi][dt_i],
                                    mybir.ActivationFunctionType.Copy,
                                )
                                nc.sync.dma_start(
                                    xhat[
                                        blk * 128 : (blk + 1) * 128,
                                        dh * 1024 + dt_i * 512 :
                                        dh * 1024 + (dt_i + 1) * 512,
                                    ],
                                    o,
                                )

    nc.finalize()
    return nc


def _split_bf16(a):
    hi = a.astype(ml_dtypes.bfloat16)
    lo = (a - hi.astype(np.float32)).astype(ml_dtypes.bfloat16)
    return hi, lo


def kernel(x, W_enc, b_enc, W_dec, b_dec):
    x = np.asarray(x, dtype=np.float32)
    W_enc = np.asarray(W_enc, dtype=np.float32)
    b_enc = np.asarray(b_enc, dtype=np.float32)
    W_dec = np.asarray(W_dec, dtype=np.float32)
    b_dec = np.asarray(b_dec, dtype=np.float32)

    aug = bool(np.any(b_enc))
    dc = D // 128 + (1 if aug else 0)
    if dc not in _cache:
        _cache[dc] = _build(dc)
    nc = _cache[dc]

    wet = np.ascontiguousarray(W_enc.T)  # [D, F] fp32
    if aug:
        pad = np.zeros((128, F), dtype=np.float32)
        pad[0] = b_enc
        wet = np.concatenate([wet, pad], axis=0)
    weh, wel = _split_bf16(wet)
    wdt = np.zeros((F + 128, D), dtype=ml_dtypes.bfloat16)
    wdt[:F] = W_dec.T.astype(ml_dtypes.bfloat16)
    wdt[F] = b_dec.astype(ml_dtypes.bfloat16)

    xs = x - b_dec[None, :]
    in_maps = []
    for c in range(NCORES):
        xt = np.ascontiguousarray(xs[c * BC : (c + 1) * BC].T)  # [D, BC]
        if aug:
            pad = np.zeros((128, BC), dtype=np.float32)
            pad[0] = 1.0
            xt = np.concatenate([xt, pad], axis=0)
        xth, xtl = _split_bf16(xt)
        in_maps.append({"xh": xth, "xl": xtl, "weh": weh, "wel": wel, "wdt": wdt})

    res = run_bass_kernel_spmd(nc, in_maps, core_ids=list(range(NCORES)))
    out = np.empty((B, D), dtype=np.float32)
    for c in range(NCORES):
        out[c * BC : (c + 1) * BC] = res.results[c]["xhat"]
    return out


# revision 9
# speedup vs baseline: 1.0275x; 1.0275x over previous
"""AutoEncoderTopK (B=4096, D=2048, F=32768, K=64) on 8 trn2 NeuronCores.

Data-parallel over batch (512 rows/core). Per core:
  P1  encode pre = (x - b_dec) @ W_enc.T + b_enc with the bf16x3 scheme
      (x_hi*w_hi + x_hi*w_lo + x_lo*w_hi, fp32 PSUM accumulate; ~1e-5 abs
      accurate -- enough for exact top-k selection at this distribution);
      relu fused into PSUM eviction; fp32 pre spilled to DRAM scratch;
      per-256-feature-chunk top-8 screen (DVE max8) -> 1024 candidates/row.
  P2  8 rounds of max8 + match_replace over candidates -> exact 64th largest
      value t* per row (values-only top-k; no indices/scatter needed).
  P3  per block-pair: re-stream pre, masked = (pre >= t*) * pre as bf16
      (split across DVE and GpSimd), PE-transpose into SBUF-resident
      maskedT half-buffers per block.
  P4  per block-pair: decode x_hat = masked @ W_dec.T + b_dec, one pass over
      W_dec.T per pair (2 blocks x 4 d-tiles = 8 PSUM accumulators).
"""

import sys

sys.path.insert(0, "/opt/trn_rl_repo")

import numpy as np
import ml_dtypes

import concourse.bacc as bacc
import concourse.mybir as mybir
import concourse.tile as tile
from concourse.bass_utils import run_bass_kernel_spmd
from concourse.masks import make_identity

F32 = mybir.dt.float32
BF16 = mybir.dt.bfloat16

B, D, F, K = 4096, 2048, 32768, 64
NCORES = 8
BC = B // NCORES            # 512 rows per core
NB = BC // 128              # 4 blocks of 128 rows
FT = F // 512               # 64 f-tiles of 512 features
NFC = F // 128              # 256 f-chunks of 128 features (decode)
CHUNK = 256                 # screening chunk
NCH = 512 // CHUNK          # max8 calls per f-tile
CAND = FT * NCH * 8         # 1024 candidates per row
DT = D // 512               # 4 d-tiles of 512 (decode output)
HFC = NFC // 2              # f-chunks per maskedT half

_cache = {}


def _build(dc):
    """dc = number of 128-row contraction chunks in encode (16, or 17 when
    b_enc is folded in via an augmented ones-row chunk)."""
    nc = bacc.Bacc()
    xh = nc.dram_tensor("xh", [dc * 128, BC], BF16, kind="ExternalInput")
    xl = nc.dram_tensor("xl", [dc * 128, BC], BF16, kind="ExternalInput")
    weh = nc.dram_tensor("weh", [dc * 128, F], BF16, kind="ExternalInput")
    wel = nc.dram_tensor("wel", [dc * 128, F], BF16, kind="ExternalInput")
    wdt = nc.dram_tensor("wdt", [F + 128, D], BF16, kind="ExternalInput")
    xhat = nc.dram_tensor("xhat", [BC, D], F32, kind="ExternalOutput")
    pre_dram = nc.dram_tensor("pre_scratch", [BC, F], F32, kind="Internal")

    with tile.TileContext(nc) as tc:
        with (
            tc.tile_pool(name="const", bufs=1) as cpool,
            tc.tile_pool(name="w", bufs=8) as wpool,
            tc.tile_pool(name="pre", bufs=4) as prepool,
            tc.tile_pool(name="small", bufs=4) as spool,
            tc.tile_pool(name="ps", bufs=8, space="PSUM") as ps,
        ):
            ident = cpool.tile([128, 128], BF16, tag="ident")
            make_identity(nc, ident)
            ones_row = cpool.tile([128, 128], BF16, tag="ones_row")
            nc.vector.memset(ones_row, 0.0)
            nc.vector.memset(ones_row[0:1, :], 1.0)
            cands = [
                cpool.tile([128, CAND], F32, tag=f"cand{b}", name=f"cand{b}")
                for b in range(NB)
            ]
            tstars = [
                cpool.tile([128, 1], F32, tag=f"tstar{b}", name=f"tstar{b}")
                for b in range(NB)
            ]

            # ---- P1: encode (bf16x3) + screen ----
            with tc.tile_pool(name="xtp", bufs=1) as xtpool:
                xh_c = []
                xl_c = []
                for c in range(dc):
                    th = xtpool.tile([128, BC], BF16, tag=f"xh{c}", name=f"xh{c}")
                    nc.sync.dma_start(th, xh[c * 128 : (c + 1) * 128, :])
                    xh_c.append(th)
                    tl = xtpool.tile([128, BC], BF16, tag=f"xl{c}", name=f"xl{c}")
                    nc.sync.dma_start(tl, xl[c * 128 : (c + 1) * 128, :])
                    xl_c.append(tl)
                for ft in range(FT):
                    psum_e = [
                        ps.tile([128, 512], F32, tag="acc", name=f"pse{ft}_{i}")
                        for i in range(NB)
                    ]
                    for c in range(dc):
                        wht = wpool.tile([128, 512], BF16, tag="w", name="wht")
                        nc.sync.dma_start(
                            wht,
                            weh[c * 128 : (c + 1) * 128, ft * 512 : (ft + 1) * 512],
                        )
                        wlt = wpool.tile([128, 512], BF16, tag="w", name="wlt")
                        nc.sync.dma_start(
                            wlt,
                            wel[c * 128 : (c + 1) * 128, ft * 512 : (ft + 1) * 512],
                        )
                        for blk in range(NB):
                            lh = xh_c[c][:, blk * 128 : (blk + 1) * 128]
                            ll = xl_c[c][:, blk * 128 : (blk + 1) * 128]
                            nc.tensor.matmul(
                                psum_e[blk], lh, wht, start=(c == 0), stop=False
                            )
                            nc.tensor.matmul(
                                psum_e[blk], lh, wlt, start=False, stop=False
                            )
                            nc.tensor.matmul(
                                psum_e[blk], ll, wht, start=False,
                                stop=(c == dc - 1),
                            )
                    for blk in range(NB):
                        pre_t = prepool.tile([128, 512], F32, tag="pre")
                        nc.scalar.activation(
                            pre_t, psum_e[blk], mybir.ActivationFunctionType.Relu
                        )
                        nc.sync.dma_start(
                            pre_dram[
                                blk * 128 : (blk + 1) * 128,
                                ft * 512 : (ft + 1) * 512,
                            ],
                            pre_t,
                        )
                        for j in range(NCH):
                            nc.vector.max(
                                out=cands[blk][
                                    :, (ft * NCH + j) * 8 : (ft * NCH + j) * 8 + 8
                                ],
                                in_=pre_t[:, j * CHUNK : (j + 1) * CHUNK],
                            )

            def emit_rounds(blk):
                # t* = 64th largest via 8 rounds of max8 + match_replace
                for r in range(K // 8):
                    m8 = spool.tile([128, 8], F32, tag="m8", name=f"m8_{blk}_{r}")
                    nc.vector.max(out=m8, in_=cands[blk])
                    if r < K // 8 - 1:
                        nc.vector.match_replace(
                            out=cands[blk],
                            in_to_replace=m8,
                            in_values=cands[blk],
                            imm_value=-1e30,
                        )
                    else:
                        nc.vector.tensor_copy(tstars[blk], m8[:, 7:8])

            LAG = 6

            def emit_p3_ft(pair, ft):
                # pre DMA + mask + 4 transposes per block of the pair
                for bi, blk in enumerate(pair):
                    pre_t = prefpool.tile(
                        [128, 512], F32, tag="pref", name=f"pref{bi}"
                    )
                    nc.sync.dma_start(
                        pre_t,
                        pre_dram[
                            blk * 128 : (blk + 1) * 128,
                            ft * 512 : (ft + 1) * 512,
                        ],
                    )
                    msk = prefpool.tile(
                        [128, 512], BF16, tag="mskf", name=f"mskf{bi}"
                    )
                    nc.vector.scalar_tensor_tensor(
                        out=msk,
                        in0=pre_t,
                        scalar=tstars[blk][:, 0:1],
                        in1=pre_t,
                        op0=mybir.AluOpType.is_ge,
                        op1=mybir.AluOpType.mult,
                    )
                    for j in range(4):
                        fcj = ft * 4 + j
                        hj, fclj = fcj // HFC, fcj % HFC
                        pt = ps.tile([128, 128], BF16, tag="acc", name="pt")
                        nc.tensor.transpose(
                            pt, msk[:, j * 128 : (j + 1) * 128], ident
                        )
                        nc.any.tensor_copy(
                            mts[bi][hj][:, fclj * 128 : (fclj + 1) * 128], pt
                        )

            # ---- P3 + P4 per block pair ----
            with (
                tc.tile_pool(name="mtp", bufs=1) as mtpool,
                tc.tile_pool(name="dec", bufs=6) as decpool,
                tc.tile_pool(name="pref", bufs=5) as prefpool,
            ):
                # maskedT half-buffers: [bi][half] so pair N+1's transposes
                # into one half overlap pair N's decode reads of the other
                mts = [
                    [
                        mtpool.tile(
                            [128, HFC * 128], BF16,
                            tag=f"mt{i}_{h}", name=f"mt{i}_{h}",
                        )
                        for h in range(2)
                    ]
                    for i in range(2)
                ]
                for pg in range(NB // 2):
                    pair = (2 * pg, 2 * pg + 1)
                    for blk in pair:
                        emit_rounds(blk)
                    psd = [
                        [
                            ps.tile(
                                [128, 512], F32, tag="acc", name=f"psd{pg}_{i}_{d}"
                            )
                            for d in range(2)
                        ]
                        for i in range(2)
                    ]
                    wa = decpool.tile([128, D], BF16, tag="wa", name="wa", bufs=1)
                    nc.sync.dma_start(wa, wdt[F : F + 128, :])
                    for dh in range(2):
                        # bias chunk opens each accumulation group
                        for bi in range(2):
                            for dt_i in range(2):
                                nc.tensor.matmul(
                                    psd[bi][dt_i],
                                    ones_row,
                                    wa[
                                        :,
                                        dh * 1024 + dt_i * 512 :
                                        dh * 1024 + (dt_i + 1) * 512,
                                    ],
                                    start=True,
                                    stop=False,
                                )
                        if dh == 0:
                            for ft in range(LAG):
                                emit_p3_ft(pair, ft)
                        for fc in range(NFC):
                            h, fcl = fc // HFC, fc % HFC
                            if dh == 0 and fc % 4 == 0 and fc // 4 + LAG < FT:
                                emit_p3_ft(pair, fc // 4 + LAG)
                            wd = decpool.tile(
                                [128, 1024], BF16, tag="wd", name="wd"
                            )
                            nc.sync.dma_start(
                                wd,
                                wdt[
                                    fc * 128 : (fc + 1) * 128,
                                    dh * 1024 : (dh + 1) * 1024,
                                ],
                            )
                            last = fc == NFC - 1
                            for bi in range(2):
                                lhsT = mts[bi][h][:, fcl * 128 : (fcl + 1) * 128]
                                for dt_i in range(2):
                                    nc.tensor.matmul(
                                        psd[bi][dt_i],
                                        lhsT,
                                        wd[:, dt_i * 512 : (dt_i + 1) * 512],
                                        start=False,
                                        stop=last,
                                    )
                        for bi, blk in enumerate(pair):
                            for dt_i in range(2):
                                o = prepool.tile([128, 512], F32, tag="o")
                                nc.scalar.activation(
                                    o,
                                    psd[bi][dt_i],
                                    mybir.ActivationFunctionType.Copy,
                                )
                                nc.sync.dma_start(
                                    xhat[
                                        blk * 128 : (blk + 1) * 128,
                                        dh * 1024 + dt_i * 512 :
                                        dh * 1024 + (dt_i + 1) * 512,
                                    ],
                                    o,
                                )

    nc.finalize()
    return nc


def _split_bf16(a):
    hi = a.astype(ml_dtypes.bfloat16)
    lo = (a - hi.astype(np.float32)).astype(ml_dtypes.bfloat16)
    return hi, lo


def kernel(x, W_enc, b_enc, W_dec, b_dec):
    x = np.asarray(x, dtype=np.float32)
    W_enc = np.asarray(W_enc, dtype=np.float32)
    b_enc = np.asarray(b_enc, dtype=np.float32)
    W_dec = np.asarray(W_dec, dtype=np.float32)
    b_dec = np.asarray(b_dec, dtype=np.float32)

    aug = bool(np.any(b_enc))
    dc = D // 128 + (1 if aug else 0)
    if dc not in _cache:
        _cache[dc] = _build(dc)
    nc = _cache[dc]

    wet = np.ascontiguousarray(W_enc.T)  # [D, F] fp32
    if aug:
        pad = np.zeros((128, F), dtype=np.float32)
        pad[0] = b_enc
        wet = np.concatenate([wet, pad], axis=0)
    weh, wel = _split_bf16(wet)
    wdt = np.zeros((F + 128, D), dtype=ml_dtypes.bfloat16)
    wdt[:F] = W_dec.T.astype(ml_dtypes.bfloat16)
    wdt[F] = b_dec.astype(ml_dtypes.bfloat16)

    xs = x - b_dec[None, :]
    in_maps = []
    for c in range(NCORES):
        xt = np.ascontiguousarray(xs[c * BC : (c + 1) * BC].T)  # [D, BC]
        if aug:
            pad = np.zeros((128, BC), dtype=np.float32)
            pad[0] = 1.0
            xt = np.concatenate([xt, pad], axis=0)
        xth, xtl = _split_bf16(xt)
        in_maps.append({"xh": xth, "xl": xtl, "weh": weh, "wel": wel, "wdt": wdt})

    res = run_bass_kernel_spmd(nc, in_maps, core_ids=list(range(NCORES)))
    out = np.empty((B, D), dtype=np.float32)
    for c in range(NCORES):
        out[c * BC : (c + 1) * BC] = res.results[c]["xhat"]
    return out
